# revision 4
# baseline (speedup 1.0000x reference)
"""MoE MLP (E=4, top-2 routing) Trainium2 kernel, 8 NeuronCores.

Strategy ("pair-group" sharding): tokens are grouped on the host by their
routed expert PAIR (6 possible pairs for E=4).  Each of the 8 cores gets one
contiguous window of tokens that all share the same expert pair (a, b), plus
the full weights of those two experts.  Each core computes
    z = p_a * gelu(x @ w1[a]) @ w2[a] + p_b * gelu(x @ w1[b]) @ w2[b] + res
for its window — entirely locally, so no collectives are needed.  The host
only permutes rows back to token order afterwards (no arithmetic on the
common path).

Tokens with !=2 routed experts are decomposed into "virtual rows" of <=2
contributions each; if the resulting group structure does not fit 8 windows
(non-top-2 routing), a dense fallback (every core: 256 tokens x all 4
experts) is used.
"""
import math
import sys

import numpy as np

try:
    import concourse.bass as bass  # noqa: F401
except Exception:
    sys.path.insert(0, "/opt/trn_rl_repo")

import concourse.bacc as bacc
import concourse.bass as bass
import concourse.mybir as mybir
import concourse.tile as tile
from concourse.bass_utils import run_bass_kernel_spmd

S, B, H, F, E = 1024, 2, 1024, 4096, 4
T = S * B
N_CORES = 8
NH = H // 128   # 8
NF = F // 128   # 32
MM_DT = mybir.dt.float32  # exact-precision debug


def _plan_windows(routing_map, probs):
    """Decompose tokens into virtual rows and pack them into 8 pure windows.

    Returns (n_slots, C, windows) where windows is a list of 8 tuples
    (experts_tuple, vrow_list); each vrow is (t, pa, pb, first).
    """
    groups = {}
    for t in range(T):
        es = np.nonzero(routing_map[t])[0]
        if len(es) == 0:
            groups.setdefault((0, 0), []).append((t, 0.0, 0.0, True))
        else:
            for k in range(0, len(es), 2):
                pair = es[k : k + 2]
                if len(pair) == 1:
                    a = b = int(pair[0])
                    pa, pb = float(probs[t, a]), 0.0
                else:
                    a, b = int(pair[0]), int(pair[1])
                    pa, pb = float(probs[t, a]), float(probs[t, b])
                groups.setdefault((a, b), []).append((t, pa, pb, k == 0))

    for C in (128, 256, 384, 512):
        if sum(math.ceil(len(g) / C) for g in groups.values()) <= N_CORES:
            windows = []
            for (a, b), lst in sorted(groups.items()):
                nparts = math.ceil(len(lst) / C)
                step = math.ceil(len(lst) / nparts)
                for i in range(nparts):
                    windows.append(((a, b), lst[i * step : (i + 1) * step]))
            while len(windows) < N_CORES:
                windows.append(((0, 0), []))
            return 2, C, windows
    # dense fallback: all 4 experts on every core, 256 tokens per core
    C = T // N_CORES
    windows = []
    for c in range(N_CORES):
        lst = [(t, 0.0, 0.0, True) for t in range(c * C, (c + 1) * C)]
        windows.append(((0, 1, 2, 3), lst))
    return E, C, windows


_NC_CACHE = {}


def _build_nc(n_slots, C):
    key = (n_slots, C)
    if key in _NC_CACHE:
        return _NC_CACHE[key]
    NT = C // 128
    f32 = mybir.dt.float32
    nc = bacc.Bacc("TRN2", target_bir_lowering=False, debug=False,
                   num_devices=N_CORES)
    xt_d = nc.declare_dram_parameter("xt", [H, C], MM_DT, isOutput=False)
    w1_d = nc.declare_dram_parameter("w1b", [n_slots, NF, 128, H], MM_DT,
                                     isOutput=False)
    w2_d = nc.declare_dram_parameter("w2b", [n_slots, F, H], MM_DT,
                                     isOutput=False)
    pp_d = nc.declare_dram_parameter("pp", [n_slots, C], f32, isOutput=False)
    res_d = nc.declare_dram_parameter("res", [C, H], f32, isOutput=False)
    out_d = nc.declare_dram_parameter("out", [C, H], f32, isOutput=True)

    with tile.TileContext(nc) as tc:
        with (
            tc.tile_pool(name="resident", bufs=1) as rpool,
            tc.tile_pool(name="w1", bufs=4) as w1pool,
            tc.tile_pool(name="w2", bufs=4) as w2pool,
            tc.tile_pool(name="abig", bufs=2) as apool,
            tc.tile_pool(name="tmp", bufs=4) as tpool,
            tc.tile_pool(name="pa", bufs=2, space="PSUM") as papool,
            tc.tile_pool(name="py", bufs=NT, space="PSUM") as pypool,
        ):
            xt_sb = rpool.tile([128, NH, C], MM_DT, tag="xt")
            nc.sync.dma_start(
                xt_sb[:], xt_d.ap().rearrange("(hc h) c -> h hc c", h=128))
            res_sb = rpool.tile([128, NT, H], f32, tag="res")
            nc.sync.dma_start(
                res_sb[:], res_d.ap().rearrange("(tc t) d -> t tc d", t=128))
            pp_sb = rpool.tile([128, n_slots, NT], f32, tag="pp")
            nc.sync.dma_start(
                pp_sb[:], pp_d.ap().rearrange("s (tc t) -> t s tc", t=128))
            z_sb = rpool.tile([128, NT, H], f32, tag="z")

            for s in range(n_slots):
                a_big = apool.tile([128, NF, C], MM_DT, tag="a")
                for Fc in range(NF):
                    w1t = w1pool.tile([128, H], MM_DT, tag="w1")
                    nc.sync.dma_start(w1t[:], w1_d[s, Fc])
                    pa = papool.tile([128, C], f32, tag="pa")
                    for Hc in range(NH):
                        nc.tensor.matmul(
                            pa[:, :],
                            w1t[:, Hc * 128:(Hc + 1) * 128],
                            xt_sb[:, Hc, :],
                            start=(Hc == 0), stop=(Hc == NH - 1))
                    nc.scalar.activation(
                        a_big[:, Fc, :], pa[:, :],
                        mybir.ActivationFunctionType.Gelu)
                for Hh in range(2):
                    psum_ys = [pypool.tile([128, 512], f32, tag="py",
                                           name=f"py_{s}_{Hh}_{i}")
                               for i in range(NT)]
                    for Fc in range(NF):
                        w2t = w2pool.tile([128, 512], MM_DT, tag="w2")
                        nc.sync.dma_start(
                            w2t[:],
                            w2_d[s, Fc * 128:(Fc + 1) * 128,
                                 Hh * 512:(Hh + 1) * 512])
                        for Tc in range(NT):
                            nc.tensor.matmul(
                                psum_ys[Tc][:, :],
                                a_big[:, Fc,
                                      Tc * 128:(Tc + 1) * 128],
                                w2t[:, :],
                                start=(Fc == 0), stop=(Fc == NF - 1))
                    for Tc in range(NT):
                        zsl = z_sb[:, Tc, Hh * 512:(Hh + 1) * 512]
                        pcol = pp_sb[:, s, Tc:Tc + 1]
                        if s == 0:
                            nc.vector.tensor_scalar(
                                zsl, psum_ys[Tc][:, :], pcol, None,
                                mybir.AluOpType.mult)
                            nc.vector.tensor_add(
                                zsl, zsl,
                                res_sb[:, Tc, Hh * 512:(Hh + 1) * 512])
                        else:
                            tmp = tpool.tile([128, 512], f32, tag="tmp")
                            nc.vector.tensor_scalar(
                                tmp[:], psum_ys[Tc][:, :], pcol, None,
                                mybir.AluOpType.mult)
                            nc.vector.tensor_add(zsl, zsl, tmp[:])
            nc.sync.dma_start(
                out_d.ap().rearrange("(tc t) d -> t tc d", t=128), z_sb[:])
    nc.compile()
    _NC_CACHE[key] = nc
    return nc


def kernel(hidden_states, mlp_residual, probs, routing_map, w1, w2,
           _trace=False):
    hidden_states = np.ascontiguousarray(np.asarray(hidden_states, np.float32))
    mlp_residual = np.ascontiguousarray(np.asarray(mlp_residual, np.float32))
    probs = np.asarray(probs, np.float32)
    routing_map = np.asarray(routing_map, bool)
    w1 = np.asarray(w1, np.float32)
    w2 = np.asarray(w2, np.float32)

    x = hidden_states.reshape(T, H)
    res = mlp_residual.reshape(T, H)
    xt_full = np.ascontiguousarray(x.T)  # [H, T]

    n_slots, C, windows = _plan_windows(routing_map, probs)
    # blocked w1 per expert: [NF, 128, H] with [Fc, h, Hc*128+f]
    w1blk = [np.ascontiguousarray(
        w1[e].reshape(NH, 128, NF, 128).transpose(2, 1, 0, 3)
        .reshape(NF, 128, H)) for e in range(E)]

    in_maps = []
    for (experts, lst) in windows:
        n = len(lst)
        tok = np.array([v[0] for v in lst], np.int64)
        xt = np.zeros((H, C), np.float32)
        if n:
            xt[:, :n] = xt_full[:, tok]
        pp = np.zeros((n_slots, C), np.float32)
        rr = np.zeros((C, H), np.float32)
        if n_slots == 2:
            if n:
                pp[0, :n] = [v[1] for v in lst]
                pp[1, :n] = [v[2] for v in lst]
                first = np.array([v[3] for v in lst], bool)
                rr[:n][first] = res[tok[first]]
        else:  # dense fallback: p = masked probs
            pp[:, :n] = (probs[tok] * routing_map[tok]).T
            rr[:n] = res[tok]
        w1b = np.stack([w1blk[e] for e in experts])
        w2b = np.stack([w2[e] for e in experts])
        in_maps.append({"xt": xt, "w1b": w1b, "w2b": w2b, "pp": pp,
                        "res": rr})

    nc = _build_nc(n_slots, C)
    r = run_bass_kernel_spmd(nc, in_maps, list(range(N_CORES)),
                             trace=_trace)

    out = np.zeros((T, H), np.float32)
    ids = np.concatenate([[v[0] for v in lst] for (_, lst) in windows
                          if lst]).astype(np.int64)
    rows = np.concatenate([r.results[c]["out"][:len(windows[c][1])]
                           for c in range(N_CORES) if windows[c][1]])
    if len(np.unique(ids)) == len(ids):
        out[ids] = rows
    else:
        np.add.at(out, ids, rows)
    result = out.reshape(S, B, H)
    if _trace:
        return result, r
    return result


# revision 5
# speedup vs baseline: 1.9967x; 1.9967x over previous
"""MoE MLP (E=4, top-2 routing) Trainium2 kernel, 8 NeuronCores.

Strategy ("pair-group" sharding): tokens are grouped on the host by their
routed expert PAIR (6 possible pairs for E=4).  Each of the 8 cores gets one
contiguous window of tokens that all share the same expert pair (a, b), plus
the full weights of those two experts.  Each core computes
    z = p_a * gelu(x @ w1[a]) @ w2[a] + p_b * gelu(x @ w1[b]) @ w2[b] + res
for its window — entirely locally, so no collectives are needed.  The host
only permutes rows back to token order afterwards (no arithmetic on the
common path).

Tokens with !=2 routed experts are decomposed into "virtual rows" of <=2
contributions each; if the resulting group structure does not fit 8 windows
(non-top-2 routing), a dense fallback (every core: 256 tokens x all 4
experts) is used.
"""
import math
import sys

import numpy as np

try:
    import concourse.bass as bass  # noqa: F401
except Exception:
    sys.path.insert(0, "/opt/trn_rl_repo")

import concourse.bacc as bacc
import concourse.bass as bass
import concourse.mybir as mybir
import concourse.tile as tile
from concourse.bass_utils import run_bass_kernel_spmd

S, B, H, F, E = 1024, 2, 1024, 4096, 4
T = S * B
N_CORES = 8
NH = H // 128   # 8
NF = F // 128   # 32
MM_DT = mybir.dt.float16  # full PE rate, ~2^-11 operand rounding
MM_NP = np.float16


def _plan_windows(routing_map, probs):
    """Decompose tokens into virtual rows and pack them into 8 pure windows.

    Returns (n_slots, C, windows) where windows is a list of 8 tuples
    (experts_tuple, vrow_list); each vrow is (t, pa, pb, first).
    """
    groups = {}
    for t in range(T):
        es = np.nonzero(routing_map[t])[0]
        if len(es) == 0:
            groups.setdefault((0, 0), []).append((t, 0.0, 0.0, True))
        else:
            for k in range(0, len(es), 2):
                pair = es[k : k + 2]
                if len(pair) == 1:
                    a = b = int(pair[0])
                    pa, pb = float(probs[t, a]), 0.0
                else:
                    a, b = int(pair[0]), int(pair[1])
                    pa, pb = float(probs[t, a]), float(probs[t, b])
                groups.setdefault((a, b), []).append((t, pa, pb, k == 0))

    for C in (128, 256, 384, 512):
        if sum(math.ceil(len(g) / C) for g in groups.values()) <= N_CORES:
            windows = []
            for (a, b), lst in sorted(groups.items()):
                nparts = math.ceil(len(lst) / C)
                step = math.ceil(len(lst) / nparts)
                for i in range(nparts):
                    windows.append(((a, b), lst[i * step : (i + 1) * step]))
            while len(windows) < N_CORES:
                windows.append(((0, 0), []))
            return 2, C, windows
    # dense fallback: all 4 experts on every core, 256 tokens per core
    C = T // N_CORES
    windows = []
    for c in range(N_CORES):
        lst = [(t, 0.0, 0.0, True) for t in range(c * C, (c + 1) * C)]
        windows.append(((0, 1, 2, 3), lst))
    return E, C, windows


_NC_CACHE = {}


def _build_nc(n_slots, C):
    key = (n_slots, C)
    if key in _NC_CACHE:
        return _NC_CACHE[key]
    NT = C // 128
    f32 = mybir.dt.float32
    nc = bacc.Bacc("TRN2", target_bir_lowering=False, debug=False,
                   num_devices=N_CORES)
    xt_d = nc.declare_dram_parameter("xt", [H, C], MM_DT, isOutput=False)
    w1_d = nc.declare_dram_parameter("w1b", [n_slots, NF, 128, H], MM_DT,
                                     isOutput=False)
    w2_d = nc.declare_dram_parameter("w2b", [n_slots, F, H], MM_DT,
                                     isOutput=False)
    pp_d = nc.declare_dram_parameter("pp", [n_slots, C], f32, isOutput=False)
    res_d = nc.declare_dram_parameter("res", [C, H], f32, isOutput=False)
    out_d = nc.declare_dram_parameter("out", [C, H], f32, isOutput=True)

    with tile.TileContext(nc) as tc:
        with (
            tc.tile_pool(name="resident", bufs=1) as rpool,
            tc.tile_pool(name="w1", bufs=4) as w1pool,
            tc.tile_pool(name="w2", bufs=4) as w2pool,
            tc.tile_pool(name="abig", bufs=2) as apool,
            tc.tile_pool(name="tmp", bufs=4) as tpool,
            tc.tile_pool(name="pa", bufs=2, space="PSUM") as papool,
            tc.tile_pool(name="py", bufs=NT, space="PSUM") as pypool,
        ):
            xt_sb = rpool.tile([128, NH, C], MM_DT, tag="xt")
            nc.sync.dma_start(
                xt_sb[:], xt_d.ap().rearrange("(hc h) c -> h hc c", h=128))
            res_sb = rpool.tile([128, NT, H], f32, tag="res")
            nc.sync.dma_start(
                res_sb[:], res_d.ap().rearrange("(tc t) d -> t tc d", t=128))
            pp_sb = rpool.tile([128, n_slots, NT], f32, tag="pp")
            nc.sync.dma_start(
                pp_sb[:], pp_d.ap().rearrange("s (tc t) -> t s tc", t=128))
            z_sb = rpool.tile([128, NT, H], f32, tag="z")

            for s in range(n_slots):
                a_big = apool.tile([128, NF, C], MM_DT, tag="a")
                for Fc in range(NF):
                    w1t = w1pool.tile([128, H], MM_DT, tag="w1")
                    nc.sync.dma_start(w1t[:], w1_d[s, Fc])
                    pa = papool.tile([128, C], f32, tag="pa")
                    for Hc in range(NH):
                        nc.tensor.matmul(
                            pa[:, :],
                            w1t[:, Hc * 128:(Hc + 1) * 128],
                            xt_sb[:, Hc, :],
                            start=(Hc == 0), stop=(Hc == NH - 1))
                    nc.scalar.activation(
                        a_big[:, Fc, :], pa[:, :],
                        mybir.ActivationFunctionType.Gelu)
                for Hh in range(2):
                    psum_ys = [pypool.tile([128, 512], f32, tag="py",
                                           name=f"py_{s}_{Hh}_{i}")
                               for i in range(NT)]
                    for Fc in range(NF):
                        w2t = w2pool.tile([128, 512], MM_DT, tag="w2")
                        nc.sync.dma_start(
                            w2t[:],
                            w2_d[s, Fc * 128:(Fc + 1) * 128,
                                 Hh * 512:(Hh + 1) * 512])
                        for Tc in range(NT):
                            nc.tensor.matmul(
                                psum_ys[Tc][:, :],
                                a_big[:, Fc,
                                      Tc * 128:(Tc + 1) * 128],
                                w2t[:, :],
                                start=(Fc == 0), stop=(Fc == NF - 1))
                    for Tc in range(NT):
                        zsl = z_sb[:, Tc, Hh * 512:(Hh + 1) * 512]
                        pcol = pp_sb[:, s, Tc:Tc + 1]
                        if s == 0:
                            nc.vector.tensor_scalar(
                                zsl, psum_ys[Tc][:, :], pcol, None,
                                mybir.AluOpType.mult)
                            nc.vector.tensor_add(
                                zsl, zsl,
                                res_sb[:, Tc, Hh * 512:(Hh + 1) * 512])
                        else:
                            tmp = tpool.tile([128, 512], f32, tag="tmp")
                            nc.vector.tensor_scalar(
                                tmp[:], psum_ys[Tc][:, :], pcol, None,
                                mybir.AluOpType.mult)
                            nc.vector.tensor_add(zsl, zsl, tmp[:])
            nc.sync.dma_start(
                out_d.ap().rearrange("(tc t) d -> t tc d", t=128), z_sb[:])
    nc.compile()
    _NC_CACHE[key] = nc
    return nc


def kernel(hidden_states, mlp_residual, probs, routing_map, w1, w2,
           _trace=False):
    hidden_states = np.ascontiguousarray(np.asarray(hidden_states, np.float32))
    mlp_residual = np.ascontiguousarray(np.asarray(mlp_residual, np.float32))
    probs = np.asarray(probs, np.float32)
    routing_map = np.asarray(routing_map, bool)
    w1 = np.asarray(w1, np.float32)
    w2 = np.asarray(w2, np.float32)

    x = hidden_states.reshape(T, H)
    res = mlp_residual.reshape(T, H)
    xt_full = np.ascontiguousarray(x.T.astype(MM_NP))  # [H, T]

    n_slots, C, windows = _plan_windows(routing_map, probs)
    # blocked w1 per expert: [NF, 128, H] with [Fc, h, Hc*128+f]
    w1blk = [np.ascontiguousarray(
        w1[e].astype(MM_NP).reshape(NH, 128, NF, 128).transpose(2, 1, 0, 3)
        .reshape(NF, 128, H)) for e in range(E)]
    w2h = w2.astype(MM_NP)

    in_maps = []
    for (experts, lst) in windows:
        n = len(lst)
        tok = np.array([v[0] for v in lst], np.int64)
        xt = np.zeros((H, C), MM_NP)
        if n:
            xt[:, :n] = xt_full[:, tok]
        pp = np.zeros((n_slots, C), np.float32)
        rr = np.zeros((C, H), np.float32)
        if n_slots == 2:
            if n:
                pp[0, :n] = [v[1] for v in lst]
                pp[1, :n] = [v[2] for v in lst]
                first = np.array([v[3] for v in lst], bool)
                rr[:n][first] = res[tok[first]]
        else:  # dense fallback: p = masked probs
            pp[:, :n] = (probs[tok] * routing_map[tok]).T
            rr[:n] = res[tok]
        w1b = np.stack([w1blk[e] for e in experts])
        w2b = np.stack([w2h[e] for e in experts])
        in_maps.append({"xt": xt, "w1b": w1b, "w2b": w2b, "pp": pp,
                        "res": rr})

    nc = _build_nc(n_slots, C)
    r = run_bass_kernel_spmd(nc, in_maps, list(range(N_CORES)),
                             trace=_trace)

    out = np.zeros((T, H), np.float32)
    ids = np.concatenate([[v[0] for v in lst] for (_, lst) in windows
                          if lst]).astype(np.int64)
    rows = np.concatenate([r.results[c]["out"][:len(windows[c][1])]
                           for c in range(N_CORES) if windows[c][1]])
    if len(np.unique(ids)) == len(ids):
        out[ids] = rows
    else:
        np.add.at(out, ids, rows)
    result = out.reshape(S, B, H)
    if _trace:
        return result, r
    return result


# revision 6
# speedup vs baseline: 58694.6847x; 29396.4437x over previous
"""MoE MLP (E=4, top-2 routing) Trainium2 kernel, 8 NeuronCores.

Strategy ("pair-group" sharding): tokens are grouped on the host by their
routed expert PAIR (6 possible pairs for E=4).  Each of the 8 cores gets one
contiguous window of tokens that all share the same expert pair (a, b), plus
the full weights of those two experts.  Each core computes
    z = p_a * gelu(x @ w1[a]) @ w2[a] + p_b * gelu(x @ w1[b]) @ w2[b] + res
for its window — entirely locally, so no collectives are needed.  The host
only permutes rows back to token order afterwards (no arithmetic on the
common path).

Tokens with !=2 routed experts are decomposed into "virtual rows" of <=2
contributions each; if the resulting group structure does not fit 8 windows
(non-top-2 routing), a dense fallback (every core: 256 tokens x all 4
experts) is used.
"""
import math
import sys

import numpy as np

try:
    import concourse.bass as bass  # noqa: F401
except Exception:
    sys.path.insert(0, "/opt/trn_rl_repo")

import concourse.bacc as bacc
import concourse.bass as bass
import concourse.mybir as mybir
import concourse.tile as tile
from concourse.bass_utils import run_bass_kernel_spmd

S, B, H, F, E = 1024, 2, 1024, 4096, 4
T = S * B
N_CORES = 8
NH = H // 128   # 8
NF = F // 128   # 32
MM_DT = mybir.dt.float16  # full PE rate, ~2^-11 operand rounding
MM_NP = np.float16


def _plan_windows(routing_map, probs):
    """Decompose tokens into virtual rows and pack them into 8 pure windows.

    Returns (n_slots, C, windows) where windows is a list of 8 tuples
    (experts_tuple, vrow_list); each vrow is (t, pa, pb, first).
    """
    groups = {}
    for t in range(T):
        es = np.nonzero(routing_map[t])[0]
        if len(es) == 0:
            groups.setdefault((0, 0), []).append((t, 0.0, 0.0, True))
        else:
            for k in range(0, len(es), 2):
                pair = es[k : k + 2]
                if len(pair) == 1:
                    a = b = int(pair[0])
                    pa, pb = float(probs[t, a]), 0.0
                else:
                    a, b = int(pair[0]), int(pair[1])
                    pa, pb = float(probs[t, a]), float(probs[t, b])
                groups.setdefault((a, b), []).append((t, pa, pb, k == 0))

    for C in (128, 256, 384, 512):
        if sum(math.ceil(len(g) / C) for g in groups.values()) <= N_CORES:
            windows = []
            for (a, b), lst in sorted(groups.items()):
                nparts = math.ceil(len(lst) / C)
                step = math.ceil(len(lst) / nparts)
                for i in range(nparts):
                    windows.append(((a, b), lst[i * step : (i + 1) * step]))
            while len(windows) < N_CORES:
                windows.append(((0, 0), []))
            return 2, C, windows
    # dense fallback: all 4 experts on every core, 256 tokens per core
    C = T // N_CORES
    windows = []
    for c in range(N_CORES):
        lst = [(t, 0.0, 0.0, True) for t in range(c * C, (c + 1) * C)]
        windows.append(((0, 1, 2, 3), lst))
    return E, C, windows


_NC_CACHE = {}


def _build_nc(n_slots, C):
    key = (n_slots, C)
    if key in _NC_CACHE:
        return _NC_CACHE[key]
    NT = C // 128
    f32 = mybir.dt.float32
    nc = bacc.Bacc("TRN2", target_bir_lowering=False, debug=False,
                   num_devices=N_CORES)
    xt_d = nc.declare_dram_parameter("xt", [H, C], MM_DT, isOutput=False)
    w1_d = nc.declare_dram_parameter("w1b", [n_slots, NF, 128, H], MM_DT,
                                     isOutput=False)
    w2_d = nc.declare_dram_parameter("w2b", [n_slots, F, H], MM_DT,
                                     isOutput=False)
    pp_d = nc.declare_dram_parameter("pp", [n_slots, C], f32, isOutput=False)
    res_d = nc.declare_dram_parameter("res", [C, H], f32, isOutput=False)
    out_d = nc.declare_dram_parameter("out", [C, H], f32, isOutput=True)

    with tile.TileContext(nc) as tc:
        with (
            tc.tile_pool(name="resident", bufs=1) as rpool,
            tc.tile_pool(name="w1", bufs=8) as w1pool,
            tc.tile_pool(name="w2", bufs=12) as w2pool,
            tc.tile_pool(name="abig", bufs=2) as apool,
            tc.tile_pool(name="tmp", bufs=4) as tpool,
            tc.tile_pool(name="pa", bufs=3, space="PSUM") as papool,
            tc.tile_pool(name="py", bufs=NT, space="PSUM") as pypool,
        ):
            xt_sb = rpool.tile([128, NH, C], MM_DT, tag="xt")
            nc.sync.dma_start(
                xt_sb[:], xt_d.ap().rearrange("(hc h) c -> h hc c", h=128))
            res_sb = rpool.tile([128, NT, H], f32, tag="res")
            nc.sync.dma_start(
                res_sb[:], res_d.ap().rearrange("(tc t) d -> t tc d", t=128))
            pp_sb = rpool.tile([128, n_slots, NT], f32, tag="pp")
            nc.sync.dma_start(
                pp_sb[:], pp_d.ap().rearrange("s (tc t) -> t s tc", t=128))
            z_sb = rpool.tile([128, NT, H], f32, tag="z")

            for s in range(n_slots):
                a_big = apool.tile([128, NF, C], MM_DT, tag="a")
                for Fc in range(NF):
                    w1t = w1pool.tile([128, H], MM_DT, tag="w1")
                    nc.sync.dma_start(w1t[:], w1_d[s, Fc])
                    pa = papool.tile([128, C], f32, tag="pa")
                    for Hc in range(NH):
                        nc.tensor.matmul(
                            pa[:, :],
                            w1t[:, Hc * 128:(Hc + 1) * 128],
                            xt_sb[:, Hc, :],
                            start=(Hc == 0), stop=(Hc == NH - 1))
                    nc.scalar.activation(
                        a_big[:, Fc, :], pa[:, :],
                        mybir.ActivationFunctionType.Gelu)
                for Hh in range(2):
                    psum_ys = [pypool.tile([128, 512], f32, tag="py",
                                           name=f"py_{s}_{Hh}_{i}")
                               for i in range(NT)]
                    for Fc in range(NF):
                        w2t = w2pool.tile([128, 512], MM_DT, tag="w2")
                        nc.sync.dma_start(
                            w2t[:],
                            w2_d[s, Fc * 128:(Fc + 1) * 128,
                                 Hh * 512:(Hh + 1) * 512])
                        for Tc in range(NT):
                            nc.tensor.matmul(
                                psum_ys[Tc][:, :],
                                a_big[:, Fc,
                                      Tc * 128:(Tc + 1) * 128],
                                w2t[:, :],
                                start=(Fc == 0), stop=(Fc == NF - 1))
                    for Tc in range(NT):
                        zsl = z_sb[:, Tc, Hh * 512:(Hh + 1) * 512]
                        pcol = pp_sb[:, s, Tc:Tc + 1]
                        if s == 0:
                            nc.vector.tensor_scalar(
                                zsl, psum_ys[Tc][:, :], pcol, None,
                                mybir.AluOpType.mult)
                            nc.vector.tensor_add(
                                zsl, zsl,
                                res_sb[:, Tc, Hh * 512:(Hh + 1) * 512])
                        else:
                            tmp = tpool.tile([128, 512], f32, tag="tmp")
                            nc.vector.tensor_scalar(
                                tmp[:], psum_ys[Tc][:, :], pcol, None,
                                mybir.AluOpType.mult)
                            nc.vector.tensor_add(zsl, zsl, tmp[:])
            nc.sync.dma_start(
                out_d.ap().rearrange("(tc t) d -> t tc d", t=128), z_sb[:])
    nc.compile()
    _NC_CACHE[key] = nc
    return nc


def kernel(hidden_states, mlp_residual, probs, routing_map, w1, w2,
           _trace=False):
    hidden_states = np.ascontiguousarray(np.asarray(hidden_states, np.float32))
    mlp_residual = np.ascontiguousarray(np.asarray(mlp_residual, np.float32))
    probs = np.asarray(probs, np.float32)
    routing_map = np.asarray(routing_map, bool)
    w1 = np.asarray(w1, np.float32)
    w2 = np.asarray(w2, np.float32)

    x = hidden_states.reshape(T, H)
    res = mlp_residual.reshape(T, H)
    xt_full = np.ascontiguousarray(x.T.astype(MM_NP))  # [H, T]

    n_slots, C, windows = _plan_windows(routing_map, probs)
    # blocked w1 per expert: [NF, 128, H] with [Fc, h, Hc*128+f]
    w1blk = [np.ascontiguousarray(
        w1[e].astype(MM_NP).reshape(NH, 128, NF, 128).transpose(2, 1, 0, 3)
        .reshape(NF, 128, H)) for e in range(E)]
    w2h = w2.astype(MM_NP)

    in_maps = []
    for (experts, lst) in windows:
        n = len(lst)
        tok = np.array([v[0] for v in lst], np.int64)
        xt = np.zeros((H, C), MM_NP)
        if n:
            xt[:, :n] = xt_full[:, tok]
        pp = np.zeros((n_slots, C), np.float32)
        rr = np.zeros((C, H), np.float32)
        if n_slots == 2:
            if n:
                pp[0, :n] = [v[1] for v in lst]
                pp[1, :n] = [v[2] for v in lst]
                first = np.array([v[3] for v in lst], bool)
                rr[:n][first] = res[tok[first]]
        else:  # dense fallback: p = masked probs
            pp[:, :n] = (probs[tok] * routing_map[tok]).T
            rr[:n] = res[tok]
        w1b = np.stack([w1blk[e] for e in experts])
        w2b = np.stack([w2h[e] for e in experts])
        in_maps.append({"xt": xt, "w1b": w1b, "w2b": w2b, "pp": pp,
                        "res": rr})

    nc = _build_nc(n_slots, C)
    r = run_bass_kernel_spmd(nc, in_maps, list(range(N_CORES)),
                             trace=_trace)

    out = np.zeros((T, H), np.float32)
    ids = np.concatenate([[v[0] for v in lst] for (_, lst) in windows
                          if lst]).astype(np.int64)
    rows = np.concatenate([r.results[c]["out"][:len(windows[c][1])]
                           for c in range(N_CORES) if windows[c][1]])
    if len(np.unique(ids)) == len(ids):
        out[ids] = rows
    else:
        np.add.at(out, ids, rows)
    result = out.reshape(S, B, H)
    if _trace:
        return result, r
    return result
